# revision 1
# baseline (speedup 1.0000x reference)
"""v2.5: valid-only [128,1] indirect gathers, length-sorted nodes, raw Block.

Nodes are sorted by degree (desc) per core so each 128-node tile only
gathers max-degree-in-tile neighbor columns (~half the slots are padding
in the unsorted layout). Raw Bass Block avoids per-call Tile sync cost.
"""
import os
import sys

for _p in ("/opt/trn_rl_repo", "/opt/pypackages"):
    if _p not in sys.path and os.path.isdir(_p):
        sys.path.append(_p)

import numpy as np

NUM_AUTHOR = 131072
D = 128
N_NODES = 32768
G = 32
NCORES = 8
NPC = N_NODES // NCORES   # 4096
P = 128
TILES = NPC // P          # 32
ZERO_ROW = NUM_AUTHOR

_CACHE = {}
LAST_RESULT = None


def _tile_maxlens(lengths):
    """Per-core sort order and per-tile gather column counts (compile-time)."""
    lengths = np.asarray(lengths).reshape(NCORES, NPC)
    orders, tlens = [], []
    for c in range(NCORES):
        order = np.argsort(-lengths[c], kind="stable")
        lens_sorted = lengths[c][order]
        lt = [max(int(lens_sorted[t * P]), 1) for t in range(TILES)]
        orders.append(order)
        tlens.append(lt)
    return orders, tlens


def _build_program(tile_lens):
    """tile_lens: [TILES] ints — max over cores of each tile's column count
    (SPMD: one program for all cores)."""
    from concourse import bacc, bass, mybir

    nc = bacc.Bacc("TRN2", target_bir_lowering=False, debug=False,
                   enable_asserts=False, num_devices=NCORES)
    dt = mybir.dt
    ctotal = sum(tile_lens)
    a2e = nc.dram_tensor("a2e", [NUM_AUTHOR + 1, D], dt.float32, kind="ExternalInput")
    idx = nc.dram_tensor("idx", [P, ctotal], dt.int32, kind="ExternalInput")
    scl = nc.dram_tensor("scl", [P, TILES], dt.float32, kind="ExternalInput")
    out = nc.dram_tensor("out", [NPC, D], dt.float32, kind="ExternalOutput")

    csum = [0]
    for L in tile_lens:
        csum.append(csum[-1] + L)

    with (
        nc.Block() as block,
        nc.sbuf_tensor("idx_sb", [P, ctotal], dt.int32) as idx_sb,
        nc.sbuf_tensor("scl_sb", [P, TILES], dt.float32) as scl_sb,
        nc.sbuf_tensor("g0", [P, G * D], dt.float32) as g0,
        nc.sbuf_tensor("g1", [P, G * D], dt.float32) as g1,
        nc.sbuf_tensor("r0", [P, D], dt.float32) as r0,
        nc.sbuf_tensor("r1", [P, D], dt.float32) as r1,
        nc.semaphore("iosem") as iosem,
        nc.semaphore("dsem0") as dsem0,
        nc.semaphore("dsem1") as dsem1,
        nc.semaphore("rsem") as rsem,
        nc.semaphore("esem") as esem,
        nc.semaphore("wsem0") as wsem0,
        nc.semaphore("wsem1") as wsem1,
    ):
        gbuf = [g0, g1]
        rbuf = [r0, r1]
        dsem = [dsem0, dsem1]
        wsem = [wsem0, wsem1]
        # cumulative gather-call counts per tile parity
        cumpar = {0: [], 1: []}
        tot = {0: 0, 1: 0}
        for t, L in enumerate(tile_lens):
            tot[t % 2] += L
            cumpar[t % 2].append(tot[t % 2])

        @block.sync
        def _(sync):
            sync.dma_start(out=idx_sb[:], in_=idx[:]).then_inc(iosem, 16)
            sync.dma_start(out=scl_sb[:], in_=scl[:]).then_inc(iosem, 16)
            for t in range(TILES):
                sync.wait_ge(rsem, t + 1)
                sync.dma_start(
                    out=out[t * P:(t + 1) * P, :], in_=rbuf[t % 2][:]
                ).then_inc(wsem[t % 2], 16)
            sync.wait_ge(wsem0, 16 * (TILES // 2))
            sync.wait_ge(wsem1, 16 * (TILES // 2))

        @block.gpsimd
        def _(gpsimd):
            gpsimd.wait_ge(iosem, 32)  # idx + scl loaded
            for t in range(TILES):
                if t >= 2:
                    gpsimd.wait_ge(rsem, t - 1)  # g[t%2] free after reduce t-2
                for j in range(tile_lens[t]):
                    c = csum[t] + j
                    gpsimd.indirect_dma_start(
                        out=gbuf[t % 2][:, j * D:(j + 1) * D],
                        out_offset=None,
                        in_=a2e[:],
                        in_offset=bass.IndirectOffsetOnAxis(
                            ap=idx_sb[:, c:c + 1], axis=0,
                        ),
                    ).then_inc(dsem[t % 2], 16)

        @block.vector
        def _(vector):
            vector.wait_ge(iosem, 32)  # scl loaded
            for t in range(TILES):
                vector.wait_ge(dsem[t % 2], 16 * cumpar[t % 2][t // 2])
                if t >= 2:
                    vector.wait_ge(wsem[t % 2], 16 * (t // 2))  # r[t%2] free
                L = tile_lens[t]
                gv = (gbuf[t % 2][:]
                      .rearrange("p (g d) -> p d g", g=G, d=D)[:, :, 0:L])
                vector.tensor_reduce(
                    out=rbuf[t % 2][:], in_=gv,
                    axis=mybir.AxisListType.X, op=mybir.AluOpType.add,
                ).then_inc(esem, 1)
                vector.wait_ge(esem, t + 1)
                sv = scl_sb[:, t:t + 1].broadcast_to([P, D])
                vector.tensor_tensor(
                    out=rbuf[t % 2][:], in0=rbuf[t % 2][:], in1=sv,
                    op=mybir.AluOpType.mult,
                ).then_inc(rsem, 1)

    nc.compile()
    return nc


def _prep_inputs(neighbors, lengths, a2e, orders, tile_lens):
    neighbors = np.asarray(neighbors).reshape(NCORES, NPC, G)
    lengths = np.asarray(lengths).reshape(NCORES, NPC)
    a2e = np.asarray(a2e, dtype=np.float32)
    ctotal = sum(tile_lens)

    idx_dram = np.full((NCORES, P, ctotal), ZERO_ROW, dtype=np.int32)
    scl_dram = np.zeros((NCORES, P, TILES), dtype=np.float32)
    for c in range(NCORES):
        order = orders[c]
        nb = neighbors[c][order]          # [NPC, G] sorted
        ln = lengths[c][order]            # [NPC]
        mask = np.arange(G)[None, :] < ln[:, None]
        nbc = np.where(mask, nb, ZERO_ROW).astype(np.int32)
        inv = np.where(ln > 0, 1.0 / np.maximum(ln, 1), 0.0).astype(np.float32)
        off = 0
        for t in range(TILES):
            L = tile_lens[t]
            idx_dram[c, :, off:off + L] = nbc[t * P:(t + 1) * P, :L]
            scl_dram[c, :, t] = inv[t * P:(t + 1) * P]
            off += L
    a2e_pad = np.concatenate([a2e, np.zeros((1, D), np.float32)], axis=0)
    return idx_dram, scl_dram, a2e_pad


def _install_ntff_hook_shim():
    import types
    if "antenv.axon_hooks" in sys.modules:
        return
    from trn_agent_boot.trn_boot import _ntff_profile_via_ctypes
    hook = _ntff_profile_via_ctypes("/opt/axon/libaxon_pjrt.so")
    mod = types.ModuleType("antenv.axon_hooks")
    mod._hook = hook
    mod.get_axon_ntff_profile_hook = lambda: mod._hook
    mod.set_axon_ntff_profile_hook = lambda h: setattr(mod, "_hook", h)
    sys.modules["antenv.axon_hooks"] = mod


def kernel(node, neighbors, lengths, a2e, _trace=False):
    global LAST_RESULT
    from concourse.bass_utils import run_bass_kernel_spmd

    if _trace:
        try:
            _install_ntff_hook_shim()
            import concourse.bass_utils as _bu
            _bu.upload_artifacts = lambda tmpdir: f"local://{tmpdir}"
        except Exception as e:
            print(f"ntff hook shim failed ({e}); running without trace")
            _trace = False

    orders, percore_lens = _tile_maxlens(lengths)
    tile_lens = [max(percore_lens[c][t] for c in range(NCORES))
                 for t in range(TILES)]
    key = tuple(tile_lens)
    if _CACHE.get("key") != key:
        _CACHE["nc"] = _build_program(tile_lens)
        _CACHE["key"] = key
    nc = _CACHE["nc"]

    idx_dram, scl_dram, a2e_pad = _prep_inputs(
        neighbors, lengths, a2e, orders, tile_lens)
    in_maps = [
        {
            "a2e": np.ascontiguousarray(a2e_pad),
            "idx": np.ascontiguousarray(idx_dram[c]),
            "scl": np.ascontiguousarray(scl_dram[c]),
        }
        for c in range(NCORES)
    ]
    res = run_bass_kernel_spmd(nc, in_maps, list(range(NCORES)), trace=_trace)
    LAST_RESULT = res

    final = np.empty((N_NODES, D), dtype=np.float32)
    for c in range(NCORES):
        block = final[c * NPC:(c + 1) * NPC]
        block[orders[c]] = res.results[c]["out"]
    return final

